# revision 1
# baseline (speedup 1.0000x reference)
"""CosineContrastiveLoss_NoExp kernel for 8 trn2 NeuronCores.

Strategy: shard the HW (=512*512) axis across the 8 cores; each core gets a
contiguous 32768-element slice of every sample as [p=128, q=256].  Inputs ship
as bf16 (half the HBM traffic of f32), interleaved per chunk into a single
DRAM tensor (one DMA instruction per chunk: DGE setup time, not bandwidth,
bounds the pipeline fill), and the binary mask rides in the LSB of in1's bf16
mantissa (zero extra traffic; <0.4% noise on x1, below bf16 rounding).

Everything the loss needs is a bilinear form over HW, computed by one
accumulating PE series with an asymmetric operand split (the stationary side
is free on the PE in the cost model; only moving columns cost cycles):
  stationary A_q [128, 97] = [ones_A | m(32) | t1=m*sq1(32) | sq1(32)]
  moving     B_q [128, 33] = [sq2(32) | ones_B]
  PSUM out[97,33] (sq = sigmoid^2):
    out[0,d]        = sum sq2[d]             (s2)
    out[1+d,d]      = sum m[d]*sq2[d]        (d2, x2 by the m encoding)
    out[33+b,b]     = sum m*sq1*sq2          (pn, x2)
    out[33+b,32]    = sum m*sq1[b]           (d1, x2)
    out[65+b,d]     = sum sq1[b]*sq2[d]      (gram G)
    out[65+b,32]    = sum sq1[b]             (s1)
Per chunk: one DMA lands x1|x2|ones_A in the slab; the mask block is built by
one DVE 4x-mode tensor_scalar, (x1.u16 & 1) << 14, writing the bf16 bit
pattern {0.0, 2.0} (no int->float convert pass; the host halves the three
mask-scaled sums); one fused ACT sigmoid reads x1|x2 and writes the
contiguous sq1|sq2 slots, whose address span overlaps no other writer (Tile's
span-based dependency tracking would otherwise serialize ACT behind the mask
work); DVE squares both blocks in place; the t1 mult and part of the squares
are split DVE/GPSIMD per chunk so both queues drain just as the sigmoid chain
ends.  Chunk sizes grow ~1.17x during the fill (DMA transfer paces the first
sigmoids), then taper so the tail chain after the last sigmoid is short.  The
matmul semaphore updates are batched (one inc on the last matmul) and the
result DMA's completion semaphore is dropped from the end-of-kernel barrier
wait (the runtime drains DMA rings at NEFF completion anyway).  Host combines
the 8 [97,33] partials and evaluates the scalar loss in f64.
"""

import os

import numpy as np

B = 32
H = 512
W = 512
HWTOT = H * W            # 262144
NCORES = 8
P = 128
QTOT = HWTOT // (NCORES * P)   # 256 q per core
# slab units per q: 0..31 raw x1 | 32..63 raw x2 | 64 ones_A (DMA'd) |
# 65..96 m | 97..128 t1 | 129..160 sq1 | 161..192 sq2 | 193 ones_B
U = 194
M = 97                   # stationary columns (u64..160)
N = 33                   # moving columns (u161..193)

# --- tuning knobs ---
QSIZES = [13, 16, 23, 31, 31, 45, 39, 32, 16, 10]
assert sum(QSIZES) == QTOT
# per-chunk units (of 32) of the t1 mult done on GPSIMD
T1_POOL = [0, 8, 16, 20, 20, 16, 15, 8, 26, 7]
# per-chunk units (of 32, per block) of the squares done on GPSIMD
SQ_POOL = [0, 0, 2, 4, 4, 8, 2, 2, 0, 0]
ONES_POOL = True         # ones-column memsets on GPSIMD
SPLIT_CHUNKS = 0         # fill chunks whose DMA+sigmoid split into x1/x2
N_WARM = 14              # PE-ramp warmup matmuls (free dim WARM_N each)
WARM_N = 256
DIRECT_PSUM_DMA = False  # DMA result straight from PSUM (skip SBUF copy)
STRIP_OUT_DMA_SEM = True

_CACHE = {}


def _build():
    import concourse.bacc as bacc
    import concourse.tile as tile
    import concourse.mybir as mybir
    from concourse.ap import AP

    f32 = mybir.dt.float32
    bf16 = mybir.dt.bfloat16
    u16 = mybir.dt.uint16
    nc = bacc.Bacc("TRN2", target_bir_lowering=False, debug=False)
    DW = 2 * B + 1   # DMA units per q: x1 | x2 | ones_A
    inb = nc.dram_tensor("inb", [P, DW * QTOT], bf16, kind="ExternalInput")
    out = nc.dram_tensor("out", [M, N], f32, kind="ExternalOutput")

    sig = mybir.ActivationFunctionType.Sigmoid
    AND = mybir.AluOpType.bitwise_and

    with tile.TileContext(nc) as tc:
        with (
            tc.tile_pool(name="big", bufs=1) as big,
            tc.tile_pool(name="psp", bufs=1, space="PSUM") as psp,
            tc.tile_pool(name="wps", bufs=1, space="PSUM") as wps,
            tc.tile_pool(name="outp", bufs=1) as outp,
        ):
            S = big.tile([P, U * QTOT], bf16)
            acc = psp.tile([M, N], f32)
            ms = nc.gpsimd if ONES_POOL else nc.vector

            # PE ramp warmup: cheap matmuls on a zeroed tile keep the PE
            # "busy" clock running so the real series runs at full rate.
            if N_WARM:
                WZ = big.tile([P, WARM_N], bf16)
                wacc = wps.tile([1, WARM_N], f32)
                ms.memset(WZ[:], 0.0)
                for _ in range(N_WARM):
                    nc.tensor.matmul(wacc[:, :], WZ[:, 0:1], WZ[:, :],
                                     start=True, stop=True)
            # warm the ACT sigmoid table while the first DMA is in flight
            warm = big.tile([1, 8], bf16)
            ms.memset(warm[:], 0.0)
            nc.scalar.activation(out=warm[:], in_=warm[:], func=sig)

            qoff = 0
            for c, qs in enumerate(QSIZES):
                # The sigmoid writes u129..192 (sq1|sq2), one regular AP
                # whose address span overlaps no other writer (Tile's
                # span-based dep tracking would otherwise serialize ACT
                # behind the GPSIMD mask work).
                Sc = S[:, U * qoff:U * (qoff + qs)].rearrange(
                    "p (u q) -> p u q", u=U)
                raw = Sc[:, 0:2 * B, :]
                m_r = Sc[:, 65:65 + B, :]
                t1_r = Sc[:, 97:97 + B, :]
                sq1_r = Sc[:, 129:129 + B, :]
                sig_r = Sc[:, 129:129 + 2 * B, :]   # sq1|sq2, contiguous
                src = inb[:][:, DW * qoff:DW * (qoff + qs)].rearrange(
                    "p (u q) -> p u q", u=DW)
                ms.memset(Sc[:, 193, :], 1.0)
                if c < SPLIT_CHUNKS:
                    # fill phase: land x1 first and sigmoid it immediately,
                    # so ACT starts half a transfer earlier
                    nc.sync.dma_start(Sc[:, 0:B, :], src[:, 0:B, :])
                    nc.sync.dma_start(Sc[:, B:DW, :], src[:, B:DW, :])
                    nc.vector.tensor_scalar(
                        m_r.bitcast(u16), raw[:, 0:B, :].bitcast(u16), 1, 14,
                        AND, mybir.AluOpType.logical_shift_left)
                    nc.scalar.activation(out=Sc[:, 129:129 + B, :],
                                         in_=raw[:, 0:B, :], func=sig)
                    nc.scalar.activation(out=Sc[:, 161:161 + B, :],
                                         in_=raw[:, B:2 * B, :], func=sig)
                else:
                    # one DMA lands raw x1 | raw x2 | ones_A
                    nc.sync.dma_start(Sc[:, 0:DW, :], src)
                    # mask block: (x & 1) << 14 writes the bf16 bit pattern
                    # {0.0, 2.0} straight from in1's LSB -- one 4x-mode pass,
                    # no int->float convert.  t1 = sq1*m is then uniformly
                    # doubled, so the host halves pn, d1 and d2.  DMA-gated
                    # only, emitted before the sigmoid.
                    nc.vector.tensor_scalar(
                        m_r.bitcast(u16), raw[:, 0:B, :].bitcast(u16), 1, 14,
                        AND, mybir.AluOpType.logical_shift_left)
                    # fused sigmoid: raw x1|x2 -> contiguous sq1|sq2 slots
                    nc.scalar.activation(out=sig_r, in_=raw[:], func=sig)
                # squares in place; GPSIMD may take head/tail units
                kq = SQ_POOL[c]
                if kq < B:
                    sq_dve = Sc[:, 129 + kq:193 - kq, :]
                    nc.vector.tensor_mul(sq_dve, sq_dve, sq_dve)
                if kq:
                    sqa = Sc[:, 129:129 + kq, :]
                    sqb = Sc[:, 193 - kq:193, :]
                    nc.gpsimd.tensor_mul(sqa, sqa, sqa)
                    nc.gpsimd.tensor_mul(sqb, sqb, sqb)
                # t1 = sq1 * m (m holds 2*mask; host compensates)
                kt = T1_POOL[c]
                if kt:
                    nc.gpsimd.tensor_mul(t1_r[:, B - kt:B, :],
                                         sq1_r[:, B - kt:B, :],
                                         m_r[:, B - kt:B, :])
                if kt < B:
                    nc.vector.tensor_mul(t1_r[:, 0:B - kt, :],
                                         sq1_r[:, 0:B - kt, :],
                                         m_r[:, 0:B - kt, :])
                for qh in range(qs):
                    q = qoff + qh
                    nc.tensor.matmul(
                        acc[:, :],
                        Sc[:, 64:64 + M, qh],
                        Sc[:, 161:161 + N, qh],
                        start=(q == 0),
                        stop=(q == QTOT - 1),
                    )
                qoff += qs
            if DIRECT_PSUM_DMA:
                nc.sync.dma_start(out[:], acc[:])
            else:
                res = outp.tile([M, N], f32)
                nc.vector.tensor_copy(res[:], acc[:])
                nc.sync.dma_start(out[:], res[:])
    _batch_matmul_sem_updates(nc)
    if STRIP_OUT_DMA_SEM:
        _strip_out_dma_sync(nc)
    _strip_dead_const_memsets(nc)
    nc.compile()
    return nc


def _strip_dead_const_memsets(nc):
    """The framework prologue memsets four small const tiles on GPSIMD
    before the all-engine entry barrier; three of them are never read by
    this kernel, and every engine waits on the barrier behind them.
    Dropping the dead ones starts the first DMA ~0.3us earlier."""
    blk = nc.m.functions[0].blocks[0]
    names = set()
    for b in nc.m.functions[0].blocks:
        for inst in b.instructions:
            for ap in list(inst.ins):
                s = str(ap)
                for n in ("const-float32-0.0", "const-float32-1.0",
                          "const-bfloat16-1.0", "const-uint8-127"):
                    if n in s:
                        names.add(n)
    keep = []
    for inst in blk.instructions:
        if (type(inst).__name__ == "InstMemset" and inst.sync_info is None
                and list(inst.outs)):
            s = str(inst.outs[0])
            if "const-" in s and not any(n in s for n in names):
                continue
        keep.append(inst)
    blk.instructions = keep
    _strip_entry_barrier(nc)


def _strip_entry_barrier(nc):
    """Drop the all-engine entry barrier in block 0.  Its only remaining job
    was ordering the const-0.0 memset (GPSIMD, block 0) before the sigmoids'
    bias reads (ACT): but GPSIMD program order already runs that memset
    before the warm-tile memsets, whose completion semaphore (155) the first
    activation waits on -- the ordering holds transitively without the
    barrier, and the first DMA no longer waits for the slowest preamble."""
    blk = nc.m.functions[0].blocks[0]
    keep = []
    for inst in blk.instructions:
        if type(inst).__name__ in ("InstDrain", "InstEventSemaphore"):
            si = inst.sync_info
            ids = set()
            if si:
                ids |= {w.id for w in si.on_wait}
                ids |= {u.id for u in si.on_update}
            if ids and ids <= {151, 152}:
                continue
        keep.append(inst)
    blk.instructions = keep


def _strip_out_dma_sync(nc):
    """The result DMA's completion semaphore only gates the end-of-kernel
    barrier (the runtime separately drains DMA rings at NEFF completion), but
    it serializes ~1.5us of sem propagation + barrier ladder after the last
    transfer.  Strip the update and relax the barrier's wait accordingly."""
    blks = nc.m.functions[0].blocks
    last_dma = None
    for blk in blks:
        for i in blk.instructions:
            if type(i).__name__ == "InstDMACopy":
                last_dma = i
    si = last_dma.sync_info
    if si is None or len(si.on_update) != 1:
        return
    upd = si.on_update[0]
    sem_id, val = upd.id, upd.update_value
    # total value the sem reaches with this update in place
    total = 0
    for blk in blks:
        for i in blk.instructions:
            s2 = i.sync_info
            if s2 is None:
                continue
            for u in s2.on_update:
                if u.id == sem_id:
                    total += u.update_value
    for blk in blks:
        for i in blk.instructions:
            s2 = i.sync_info
            if s2 is None or i is last_dma:
                continue
            changed = False
            for w in s2.on_wait:
                if w.id == sem_id and w.wait_value == total:
                    w.wait_value = total - val
                    changed = True
            if changed:
                i.sync_info = s2


def _batch_matmul_sem_updates(nc):
    """Tile emits a +1 sem-inc on every matmul, but the only consumers wait
    for the final value.  Strip the per-instruction updates (sequencer sem
    writes serialize at ~26-100ns each) and retarget the waiters to the
    reduced final count."""
    for blk in nc.m.functions[0].blocks:
        mms = [i for i in blk.instructions if type(i).__name__ == "InstMatmult"]
        if not mms:
            continue
        total = 0
        sem_id = None
        for i in mms:
            si = i.sync_info
            if si is None:
                continue
            for u in si.on_update:
                assert u.update_mode == "sem-inc"
                sem_id = u.id
                total += u.update_value
        kept = 0
        for i in mms[:-1]:
            si = i.sync_info
            if si is None:
                continue
            if len(si.on_wait) == 0 and len(si.on_update) == 1:
                i.sync_info = None
            else:
                kept += sum(u.update_value for u in si.on_update
                            if u.id == sem_id)
        kept += 1  # the last matmul keeps its +1
        for blk2 in nc.m.functions[0].blocks:
            for i in blk2.instructions:
                si = i.sync_info
                if si is None:
                    continue
                changed = False
                for w in si.on_wait:
                    if w.id == sem_id and w.wait_value == total:
                        w.wait_value = kept
                        changed = True
                if changed:
                    i.sync_info = si


def _get_nc():
    if "nc" not in _CACHE:
        _CACHE["nc"] = _build()
    return _CACHE["nc"]


def _stage(b1, b2):
    """Two [B, HWTOT] bf16 arrays -> per-core [P, (2B+1)*QTOT] interleaved
    chunk-major: per chunk the x1 block, the x2 block, then a ones unit."""
    ones = np.ones((NCORES, P, 1, QTOT), dtype=b1.dtype)
    v1 = b1.reshape(B, NCORES, P, QTOT)
    v2 = b2.reshape(B, NCORES, P, QTOT)
    parts = []
    qoff = 0
    for qs in QSIZES:
        k1 = v1[..., qoff:qoff + qs].transpose(1, 2, 0, 3)  # [NC, P, B, qs]
        k2 = v2[..., qoff:qoff + qs].transpose(1, 2, 0, 3)
        parts.append(k1.reshape(NCORES, P, B * qs))
        parts.append(k2.reshape(NCORES, P, B * qs))
        parts.append(ones[..., qoff:qoff + qs].reshape(NCORES, P, qs))
        qoff += qs
    outv = np.ascontiguousarray(np.concatenate(parts, axis=2))
    return [outv[k] for k in range(NCORES)]


LAST_RESULT = None


def kernel(input1, input2, mask):
    import ml_dtypes
    from concourse.bass_utils import run_bass_kernel_spmd

    global LAST_RESULT
    x1 = np.asarray(input1, dtype=np.float32).reshape(B, HWTOT)
    x2 = np.asarray(input2, dtype=np.float32).reshape(B, HWTOT)
    mk = (np.asarray(mask, dtype=np.float32).reshape(B, HWTOT) != 0)

    b1 = x1.astype(ml_dtypes.bfloat16)
    u1 = b1.view(np.uint16)
    u1 = (u1 & np.uint16(0xFFFE)) | mk.astype(np.uint16)
    b1 = u1.view(ml_dtypes.bfloat16)
    b2 = x2.astype(ml_dtypes.bfloat16)

    sb = _stage(b1, b2)
    in_maps = [{"inb": sb[k]} for k in range(NCORES)]
    nc = _get_nc()
    trace = bool(int(os.environ.get("BASSKERNEL_TRACE", "0")))
    try:
        res = run_bass_kernel_spmd(
            nc, in_maps, core_ids=list(range(NCORES)), trace=trace,
        )
    except ModuleNotFoundError:
        res = run_bass_kernel_spmd(
            nc, in_maps, core_ids=list(range(NCORES)), trace=False,
        )
    LAST_RESULT = res

    Ms = np.zeros((M, N), dtype=np.float64)
    for r in res.results:
        Ms += np.asarray(r["out"], dtype=np.float64)

    s2v = Ms[0, 0:B]
    # the m block holds 2*mask, so d2 and the t1-derived sums are doubled
    d2 = 0.5 * np.diag(Ms[1:1 + B, 0:B])
    pn = 0.5 * np.diag(Ms[33:33 + B, 0:B])
    d1 = 0.5 * Ms[33:33 + B, B]
    G = Ms[65:65 + B, 0:B]
    s1v = Ms[65:65 + B, B]

    sim_pos = np.sqrt(pn) / (np.sqrt(d1) * np.sqrt(d2))          # [B]
    sim = np.sqrt(G) / (np.sqrt(s1v)[:, None] * np.sqrt(s2v)[None, :])
    sim_neg = sim.sum(axis=1) - np.diag(sim)                      # [B]
    ratio = sim_pos[None, :] / (sim_pos[None, :] + sim_neg[:, None])
    loss = -np.log(ratio)
    return np.array(loss.mean(), dtype=np.float32)



# revision 2
# speedup vs baseline: 1.0307x; 1.0307x over previous
"""CosineContrastiveLoss_NoExp kernel for 8 trn2 NeuronCores — v3 (fp8).

Same bilinear-form architecture as the bf16 baseline, but inputs ship as
fp8-e4m3 (mask in x1's LSB): HBM traffic halves again (64B per q per
partition), so the DMA stream outruns the ACT sigmoid stream by >2x and the
sigmoid can run in ~7 large instructions instead of ~10 paced ones.

The fp8 mask trick needs the m block extracted from byte PAIRS (u16 view of
the fp8 bytes): two 4x-mode tensor_scalar passes produce the mask in an
evens-then-odds ("halves") pixel order per chunk unit.  The sigmoid itself
re-orders to match at zero cost: ACT charges free-size only, so its input AP
reads x strided [u, parity, j] while the output lands contiguous — every
downstream block (m, t1, sq1, sq2) then shares the same halves pixel order
and all DVE ops keep their fast modes.  (All reductions are over pixels, so
a per-chunk pixel relabeling is harmless.)

Slab layout per q (bf16 work tile): m(32) | t1(32) | sq1(32) | ones_A |
ones_B | sq2(32); stationary = [m|t1|sq1|ones_A] (97 cols), moving =
[ones_B|sq2] (33).  ones_A/ones_B are adjacent so one Pool memset per chunk
covers both.  A separate fp8 tile holds the raw DMA'd x1|x2 bytes.
"""

import os

import numpy as np

B = 32
H = 512
W = 512
HWTOT = H * W            # 262144
NCORES = 8
P = 128
QTOT = HWTOT // (NCORES * P)   # 256 q per core
# bf16 slab units per q: 0..31 m | 32..63 t1 | 64 ones_A | 65..96 sq1 |
# 97..128 sq2 | 129 ones_B  (sq1|sq2 adjacent -> one fused sigmoid out span)
UW = 130
M = 97                   # stationary columns (u0..96)
N = 33                   # moving columns (u97..129)
RU = 2 * B               # raw fp8 units per q: x1(32) | x2(32)

# --- tuning knobs ---
QSIZES = [14, 24, 54, 52, 44, 24, 20, 16, 8]
assert sum(QSIZES) == QTOT
SQ_POOL = [0, 0, 0, 0, 2, 0, 0, 8, 3]
T1_POOL = [0, 4, 10, 12, 18, 12, 0, 10, 2]
N_WARM = 0
WARM_N = 256
SPLIT_FROM = 9           # tail chunks with separate x1/x2 sigmoid phases (9 = none)

_CACHE = {}


def _build(qsizes=None, sq_pool=None, t1_pool=None, n_warm=None,
           split_from=None):
    import concourse.bacc as bacc
    import concourse.tile as tile
    import concourse.mybir as mybir

    qsizes = QSIZES if qsizes is None else qsizes
    sq_pool = SQ_POOL if sq_pool is None else sq_pool
    t1_pool = T1_POOL if t1_pool is None else t1_pool
    n_warm = N_WARM if n_warm is None else n_warm
    split_from = SPLIT_FROM if split_from is None else split_from
    assert sum(qsizes) == QTOT
    assert all(qs % 2 == 0 for qs in qsizes)

    f32 = mybir.dt.float32
    bf16 = mybir.dt.bfloat16
    fp8 = mybir.dt.float8e4
    u16 = mybir.dt.uint16
    nc = bacc.Bacc("TRN2", target_bir_lowering=False, debug=False)
    inb = nc.dram_tensor("inb", [P, RU * QTOT], fp8, kind="ExternalInput")
    out = nc.dram_tensor("out", [M, N], f32, kind="ExternalOutput")

    sig = mybir.ActivationFunctionType.Sigmoid
    AND = mybir.AluOpType.bitwise_and
    SHL = mybir.AluOpType.logical_shift_left

    with tile.TileContext(nc) as tc:
        with (
            tc.tile_pool(name="big", bufs=1) as big,
            tc.tile_pool(name="psp", bufs=1, space="PSUM") as psp,
            tc.tile_pool(name="wps", bufs=1, space="PSUM") as wps,
            tc.tile_pool(name="outp", bufs=1) as outp,
        ):
            R = big.tile([P, RU * QTOT], fp8)
            S = big.tile([P, UW * QTOT], bf16)
            acc = psp.tile([M, N], f32)

            if n_warm:
                WZ = big.tile([P, WARM_N], bf16)
                wacc = wps.tile([1, WARM_N], f32)
                nc.gpsimd.memset(WZ[:], 0.0)
                for _ in range(n_warm):
                    nc.tensor.matmul(wacc[:, :], WZ[:, 0:1], WZ[:, :],
                                     start=True, stop=True)
            # warm the ACT sigmoid table while the first DMA is in flight
            warm = big.tile([1, 8], bf16)
            nc.gpsimd.memset(warm[:], 0.0)
            nc.scalar.activation(out=warm[:], in_=warm[:], func=sig)

            nch = len(qsizes)
            # Loop A: per chunk, everything that is DMA-gated only — the
            # ones memset (one strided op covers ones_A u64 and ones_B u129,
            # 65 units apart), the input DMA, and the two 4x m-build passes.
            # Emitting the m work here (ahead of all sigmoid consumers) lets
            # DVE burn through it during its early starved phase instead of
            # queuing it between the paced sq/t1 ops.
            qoff = 0
            for c, qs in enumerate(qsizes):
                So = S[:, UW * qoff:UW * (qoff + qs)].rearrange(
                    "p (uu u q) -> p uu u q", uu=2, u=65)
                nc.gpsimd.memset(So[:, :, 64, :], 1.0)
                Sc = S[:, UW * qoff:UW * (qoff + qs)].rearrange(
                    "p (u q) -> p u q", u=UW)
                Rc = R[:, RU * qoff:RU * (qoff + qs)]
                nc.sync.dma_start(Rc, inb[:][:, RU * qoff:RU * (qoff + qs)])
                # mask from x1's fp8 LSBs, via the u16 pair view: two 4x
                # passes write bf16 {0,2} into the halves pixel order that
                # the strided sigmoid also produces.
                pair = Rc[:, 0:B * qs].bitcast(u16).rearrange(
                    "p (u j) -> p u j", u=B)
                mh = Sc[:, 0:B, :].rearrange("p u (t j) -> p u t j", t=2)
                nc.vector.tensor_scalar(
                    mh[:, :, 0, :].bitcast(u16), pair, 1, 14, AND, SHL)
                nc.vector.tensor_scalar(
                    mh[:, :, 1, :].bitcast(u16), pair, 256, 6, AND, SHL)
                qoff += qs
            offs = []
            qoff = 0
            for qs in qsizes:
                offs.append(qoff)
                qoff += qs

            def emit_x1(c):
                # x1 sigmoid + sq1 + t1 for chunk c (tail phase 2)
                qs, qo = qsizes[c], offs[c]
                Sc = S[:, UW * qo:UW * (qo + qs)].rearrange(
                    "p (u q) -> p u q", u=UW)
                Rc = R[:, RU * qo:RU * (qo + qs)]
                m_r = Sc[:, 0:B, :]
                t1_r = Sc[:, B:2 * B, :]
                sq1_r = Sc[:, 65:65 + B, :]
                r1 = Rc[:, 0:B * qs].rearrange(
                    "p (u j t) -> p u t j", u=B, t=2)
                s1 = sq1_r.rearrange("p u (t j) -> p u t j", t=2)
                nc.scalar.activation(out=s1, in_=r1, func=sig)
                kq, kt = sq_pool[c], t1_pool[c]
                if kq:
                    sqa = Sc[:, 65:65 + kq, :]
                    nc.gpsimd.tensor_mul(sqa, sqa, sqa)
                sq1d = Sc[:, 65 + kq:97, :]
                nc.vector.tensor_mul(sq1d, sq1d, sq1d)
                if kt:
                    nc.gpsimd.tensor_mul(t1_r[:, B - kt:B, :],
                                         sq1_r[:, B - kt:B, :],
                                         m_r[:, B - kt:B, :])
                if kt < B:
                    nc.vector.tensor_mul(t1_r[:, 0:B - kt, :],
                                         sq1_r[:, 0:B - kt, :],
                                         m_r[:, 0:B - kt, :])

            def emit_x2_mm(c):
                # x2 sigmoid + sq2 + matmuls for chunk c (tail phase 3)
                qs, qo = qsizes[c], offs[c]
                Sc = S[:, UW * qo:UW * (qo + qs)].rearrange(
                    "p (u q) -> p u q", u=UW)
                Rc = R[:, RU * qo:RU * (qo + qs)]
                r2 = Rc[:, B * qs:2 * B * qs].rearrange(
                    "p (u j t) -> p u t j", u=B, t=2)
                s2 = Sc[:, 97:97 + B, :].rearrange("p u (t j) -> p u t j", t=2)
                nc.scalar.activation(out=s2, in_=r2, func=sig)
                sq2d = Sc[:, 97:129, :]
                nc.vector.tensor_mul(sq2d, sq2d, sq2d)
                for qh in range(qs):
                    q = qo + qh
                    nc.tensor.matmul(
                        acc[:, :], Sc[:, 0:M, qh], Sc[:, M:M + N, qh],
                        start=(q == 0), stop=(q == QTOT - 1))

            for c, qs in enumerate(qsizes):
                qoff = offs[c]
                if c >= split_from:
                    continue
                Sc = S[:, UW * qoff:UW * (qoff + qs)].rearrange(
                    "p (u q) -> p u q", u=UW)
                Rc = R[:, RU * qoff:RU * (qoff + qs)]
                m_r = Sc[:, 0:B, :]
                t1_r = Sc[:, B:2 * B, :]
                sq1_r = Sc[:, 65:65 + B, :]
                sq2_r = Sc[:, 97:97 + B, :]
                kq = sq_pool[c]
                kt = t1_pool[c]
                # fused: x1|x2 sigmoid in ONE ACT op — the input reads fp8
                # pairs strided so the bf16 output lands in the same halves
                # pixel order as the m block (free: ACT charges free-size
                # only, not strides); out spans the adjacent sq1|sq2 blocks.
                rall = Rc[:, 0:RU * qs].rearrange(
                    "p (u j t) -> p u t j", u=RU, t=2)
                sall = Sc[:, 65:129, :].rearrange(
                    "p u (t j) -> p u t j", t=2)
                nc.scalar.activation(out=sall, in_=rall, func=sig)
                if kq:
                    sqa = Sc[:, 65:65 + kq, :]
                    sqb = Sc[:, 129 - kq:129, :]
                    nc.gpsimd.tensor_mul(sqa, sqa, sqa)
                    nc.gpsimd.tensor_mul(sqb, sqb, sqb)
                sqd = Sc[:, 65 + kq:129 - kq, :]
                nc.vector.tensor_mul(sqd, sqd, sqd)
                if kt:
                    nc.gpsimd.tensor_mul(t1_r[:, B - kt:B, :],
                                         sq1_r[:, B - kt:B, :],
                                         m_r[:, B - kt:B, :])
                if kt < B:
                    nc.vector.tensor_mul(t1_r[:, 0:B - kt, :],
                                         sq1_r[:, 0:B - kt, :],
                                         m_r[:, 0:B - kt, :])
                for qh in range(qs):
                    q = qoff + qh
                    nc.tensor.matmul(
                        acc[:, :],
                        Sc[:, 0:M, qh],
                        Sc[:, M:M + N, qh],
                        start=(q == 0),
                        stop=(q == QTOT - 1),
                    )
            # tail: x1 sigmoids (+sq1/t1) for all tail chunks first, then the
            # x2 sigmoids (+sq2/matmuls) — after the final ACT op only one
            # small sq2 remains in the drain.
            for c in range(split_from, nch):
                emit_x1(c)
            for c in range(split_from, nch):
                emit_x2_mm(c)
            res = outp.tile([M, N], f32)
            nc.vector.tensor_copy(res[:], acc[:])
            nc.sync.dma_start(out[:], res[:])
    _batch_matmul_sem_updates(nc)
    _strip_same_engine_sem_waits(nc)
    _strip_out_dma_sync(nc)
    _strip_dead_const_memsets(nc)
    nc.compile()
    return nc


def _strip_same_engine_sem_waits(nc):
    """Tile guards RAW deps between instructions on the SAME engine with that
    engine's completion semaphore, but compute engines execute their stream
    in order (the DVE drains its pipe between ops), so those waits only add
    the producer's pipeline-ack + sem-propagation latency (~95ns per chained
    dep).  Drop any wait on a semaphore that is (a) only ever updated by
    instructions of the waiting instruction's own engine, and (b) already
    guaranteed by program order (wait_value <= updates emitted so far)."""
    blks = nc.m.functions[0].blocks
    owner = {}   # sem id -> set of engines updating it
    for blk in blks:
        for i in blk.instructions:
            si = i.sync_info
            if si is None:
                continue
            for u in si.on_update:
                owner.setdefault(u.id, set()).add(str(i.engine))
    skip = {"InstDMACopy", "InstDMAScatterAddAnt", "InstTriggerDma",
            "InstDmaTransposeAnt"}
    for blk in blks:
        done = {}  # sem id -> updates emitted so far by its (sole) engine
        for i in blk.instructions:
            si = i.sync_info
            if si is None:
                continue
            eng = str(i.engine)
            if type(i).__name__ not in skip:
                new_waits = []
                for w in si.on_wait:
                    own = owner.get(w.id)
                    if (own == {eng} and w.wait_value is not None
                            and w.wait_value <= done.get(w.id, 0)):
                        continue
                    new_waits.append(w)
                if len(new_waits) != len(si.on_wait):
                    si.on_wait = new_waits
                    i.sync_info = si
            for u in si.on_update:
                if owner.get(u.id) == {eng} and u.update_value is not None:
                    done[u.id] = done.get(u.id, 0) + u.update_value


def _strip_dead_const_memsets(nc):
    blk = nc.m.functions[0].blocks[0]
    names = set()
    for b in nc.m.functions[0].blocks:
        for inst in b.instructions:
            for ap in list(inst.ins):
                s = str(ap)
                for n in ("const-float32-0.0", "const-float32-1.0",
                          "const-bfloat16-1.0", "const-uint8-127"):
                    if n in s:
                        names.add(n)
    keep = []
    for inst in blk.instructions:
        if (type(inst).__name__ == "InstMemset" and inst.sync_info is None
                and list(inst.outs)):
            s = str(inst.outs[0])
            if "const-" in s and not any(n in s for n in names):
                continue
        keep.append(inst)
    blk.instructions = keep
    _strip_entry_barrier(nc)


def _strip_entry_barrier(nc):
    blk = nc.m.functions[0].blocks[0]
    barrier_ids = set()
    for inst in blk.instructions:
        if type(inst).__name__ in ("InstDrain", "InstEventSemaphore"):
            si = inst.sync_info
            if si:
                barrier_ids |= {w.id for w in si.on_wait}
                barrier_ids |= {u.id for u in si.on_update}
    keep = []
    for inst in blk.instructions:
        if type(inst).__name__ in ("InstDrain", "InstEventSemaphore"):
            si = inst.sync_info
            ids = set()
            if si:
                ids |= {w.id for w in si.on_wait}
                ids |= {u.id for u in si.on_update}
            if ids and ids <= barrier_ids:
                continue
            if si is None and type(inst).__name__ == "InstDrain":
                continue
        keep.append(inst)
    blk.instructions = keep


def _strip_out_dma_sync(nc):
    blks = nc.m.functions[0].blocks
    last_dma = None
    for blk in blks:
        for i in blk.instructions:
            if type(i).__name__ == "InstDMACopy":
                last_dma = i
    si = last_dma.sync_info
    if si is None or len(si.on_update) != 1:
        return
    upd = si.on_update[0]
    sem_id, val = upd.id, upd.update_value
    total = 0
    for blk in blks:
        for i in blk.instructions:
            s2 = i.sync_info
            if s2 is None:
                continue
            for u in s2.on_update:
                if u.id == sem_id:
                    total += u.update_value
    for blk in blks:
        for i in blk.instructions:
            s2 = i.sync_info
            if s2 is None or i is last_dma:
                continue
            changed = False
            for w in s2.on_wait:
                if w.id == sem_id and w.wait_value == total:
                    w.wait_value = total - val
                    changed = True
            if changed:
                i.sync_info = s2


def _batch_matmul_sem_updates(nc):
    for blk in nc.m.functions[0].blocks:
        mms = [i for i in blk.instructions if type(i).__name__ == "InstMatmult"]
        if not mms:
            continue
        total = 0
        sem_id = None
        for i in mms:
            si = i.sync_info
            if si is None:
                continue
            for u in si.on_update:
                assert u.update_mode == "sem-inc"
                sem_id = u.id
                total += u.update_value
        kept = 0
        for i in mms[:-1]:
            si = i.sync_info
            if si is None:
                continue
            if len(si.on_wait) == 0 and len(si.on_update) == 1:
                i.sync_info = None
            else:
                kept += sum(u.update_value for u in si.on_update
                            if u.id == sem_id)
        kept += 1
        for blk2 in nc.m.functions[0].blocks:
            for i in blk2.instructions:
                si = i.sync_info
                if si is None:
                    continue
                changed = False
                for w in si.on_wait:
                    if w.id == sem_id and w.wait_value == total:
                        w.wait_value = kept
                        changed = True
                if changed:
                    i.sync_info = si


def _get_nc():
    if "nc" not in _CACHE:
        _CACHE["nc"] = _build()
    return _CACHE["nc"]


_F8LUT = None


def _f8_lut():
    global _F8LUT
    if _F8LUT is None:
        import ml_dtypes
        _F8LUT = np.arange(256, dtype=np.uint8).view(
            ml_dtypes.float8_e4m3fn).astype(np.float32)
    return _F8LUT


def _to_fp8_mask_lsb(x, mask_bits):
    """fp8-e4m3 bytes for x with the LSB forced to mask_bits, choosing the
    closer of the two LSB-matching neighbours."""
    import ml_dtypes
    lut = _f8_lut()
    a = x.astype(ml_dtypes.float8_e4m3fn).view(np.uint8)
    c0 = (a & np.uint8(0xFE)) | mask_bits
    # alternate candidate two code-points away (same LSB), toward x
    v0 = lut[c0]
    mag_up = (np.abs(x) > np.abs(v0))
    step = np.where(mag_up, 2, -2).astype(np.int16)
    c1 = (c0.astype(np.int16) + step)
    # keep sign bit intact; clamp to valid magnitude range
    bad = (c1 & 0x7F) < 0
    c1 = np.where((c1 ^ c0.astype(np.int16)) & 0x80, c0.astype(np.int16), c1)
    c1 = np.where(bad, c0.astype(np.int16), c1).astype(np.uint8)
    v1 = lut[c1]
    pick1 = np.abs(v1 - x) < np.abs(v0 - x)
    return np.where(pick1, c1, c0).astype(np.uint8)


def _stage(u1, u2, qsizes=None):
    """Two [B, HWTOT] u8 (fp8-byte) arrays -> per-core [P, 2B*QTOT] chunk-
    major: per chunk the x1 block then the x2 block."""
    qsizes = QSIZES if qsizes is None else qsizes
    v1 = u1.reshape(B, NCORES, P, QTOT)
    v2 = u2.reshape(B, NCORES, P, QTOT)
    parts = []
    qoff = 0
    for qs in qsizes:
        k1 = v1[..., qoff:qoff + qs].transpose(1, 2, 0, 3)  # [NC, P, B, qs]
        k2 = v2[..., qoff:qoff + qs].transpose(1, 2, 0, 3)
        parts.append(k1.reshape(NCORES, P, B * qs))
        parts.append(k2.reshape(NCORES, P, B * qs))
        qoff += qs
    outv = np.ascontiguousarray(np.concatenate(parts, axis=2))
    return [outv[k] for k in range(NCORES)]


LAST_RESULT = None


def kernel(input1, input2, mask):
    import ml_dtypes
    from concourse.bass_utils import run_bass_kernel_spmd

    global LAST_RESULT
    x1 = np.asarray(input1, dtype=np.float32).reshape(B, HWTOT)
    x2 = np.asarray(input2, dtype=np.float32).reshape(B, HWTOT)
    mk = (np.asarray(mask, dtype=np.float32).reshape(B, HWTOT) != 0)

    u1 = _to_fp8_mask_lsb(x1, mk.astype(np.uint8))
    u2 = x2.astype(ml_dtypes.float8_e4m3fn).view(np.uint8)

    sb = _stage(u1, u2)
    in_maps = [{"inb": sb[k].view(ml_dtypes.float8_e4m3fn)}
               for k in range(NCORES)]
    nc = _get_nc()
    trace = bool(int(os.environ.get("BASSKERNEL_TRACE", "0")))
    try:
        res = run_bass_kernel_spmd(
            nc, in_maps, core_ids=list(range(NCORES)), trace=trace,
        )
    except ModuleNotFoundError:
        res = run_bass_kernel_spmd(
            nc, in_maps, core_ids=list(range(NCORES)), trace=False,
        )
    LAST_RESULT = res

    Ms = np.zeros((M, N), dtype=np.float64)
    for r in res.results:
        Ms += np.asarray(r["out"], dtype=np.float64)

    # stationary rows: m(0..31) | t1(32..63) | ones_A(64) | sq1(65..96)
    # moving cols:     sq2(0..31) | ones_B(32)
    d2 = 0.5 * np.diag(Ms[0:B, 0:B])
    pn = 0.5 * np.diag(Ms[B:2 * B, 0:B])
    d1 = 0.5 * Ms[B:2 * B, B]
    s2v = Ms[64, 0:B]
    G = Ms[65:65 + B, 0:B]
    s1v = Ms[65:65 + B, B]

    sim_pos = np.sqrt(pn) / (np.sqrt(d1) * np.sqrt(d2))          # [B]
    sim = np.sqrt(G) / (np.sqrt(s1v)[:, None] * np.sqrt(s2v)[None, :])
    sim_neg = sim.sum(axis=1) - np.diag(sim)                      # [B]
    ratio = sim_pos[None, :] / (sim_pos[None, :] + sim_neg[:, None])
    loss = -np.log(ratio)
    return np.array(loss.mean(), dtype=np.float32)


# revision 3
# speedup vs baseline: 1.0338x; 1.0030x over previous
"""CosineContrastiveLoss_NoExp kernel for 8 trn2 NeuronCores — v3 (fp8).

Same bilinear-form architecture as the bf16 baseline, but inputs ship as
fp8-e4m3 (mask in x1's LSB): HBM traffic halves again (64B per q per
partition), so the DMA stream outruns the ACT sigmoid stream by >2x and the
sigmoid can run in ~7 large instructions instead of ~10 paced ones.

The fp8 mask trick needs the m block extracted from byte PAIRS (u16 view of
the fp8 bytes): two 4x-mode tensor_scalar passes produce the mask in an
evens-then-odds ("halves") pixel order per chunk unit.  The sigmoid itself
re-orders to match at zero cost: ACT charges free-size only, so its input AP
reads x strided [u, parity, j] while the output lands contiguous — every
downstream block (m, t1, sq1, sq2) then shares the same halves pixel order
and all DVE ops keep their fast modes.  (All reductions are over pixels, so
a per-chunk pixel relabeling is harmless.)

Slab layout per q (bf16 work tile): m(32) | t1(32) | sq1(32) | ones_A |
ones_B | sq2(32); stationary = [m|t1|sq1|ones_A] (97 cols), moving =
[ones_B|sq2] (33).  ones_A/ones_B are adjacent so one Pool memset per chunk
covers both.  A separate fp8 tile holds the raw DMA'd x1|x2 bytes.
"""

import os

import numpy as np

B = 32
H = 512
W = 512
HWTOT = H * W            # 262144
NCORES = 8
P = 128
QTOT = HWTOT // (NCORES * P)   # 256 q per core
# bf16 slab units per q: 0..31 m | 32..63 t1 | 64 ones_A | 65..96 sq1 |
# 97..128 sq2 | 129 ones_B  (sq1|sq2 adjacent -> one fused sigmoid out span)
UW = 130
M = 97                   # stationary columns (u0..96)
N = 33                   # moving columns (u97..129)
RU = 2 * B               # raw fp8 units per q: x1(32) | x2(32)

# --- tuning knobs ---
QSIZES = [14, 22, 48, 52, 44, 30, 22, 14, 10]
assert sum(QSIZES) == QTOT
SQ_POOL = [0, 0, 1, 0, 2, 0, 0, 6, 7]
T1_POOL = [0, 10, 14, 12, 18, 12, 0, 14, 2]
N_WARM = 0
WARM_N = 256
SPLIT_FROM = 9           # tail chunks with separate x1/x2 sigmoid phases (9 = none)

_CACHE = {}


def _build(qsizes=None, sq_pool=None, t1_pool=None, n_warm=None,
           split_from=None):
    import concourse.bacc as bacc
    import concourse.tile as tile
    import concourse.mybir as mybir

    qsizes = QSIZES if qsizes is None else qsizes
    sq_pool = SQ_POOL if sq_pool is None else sq_pool
    t1_pool = T1_POOL if t1_pool is None else t1_pool
    n_warm = N_WARM if n_warm is None else n_warm
    split_from = SPLIT_FROM if split_from is None else split_from
    assert sum(qsizes) == QTOT
    assert all(qs % 2 == 0 for qs in qsizes)

    f32 = mybir.dt.float32
    bf16 = mybir.dt.bfloat16
    fp8 = mybir.dt.float8e4
    u16 = mybir.dt.uint16
    nc = bacc.Bacc("TRN2", target_bir_lowering=False, debug=False)
    inb = nc.dram_tensor("inb", [P, RU * QTOT], fp8, kind="ExternalInput")
    out = nc.dram_tensor("out", [M, N], f32, kind="ExternalOutput")

    sig = mybir.ActivationFunctionType.Sigmoid
    AND = mybir.AluOpType.bitwise_and
    SHL = mybir.AluOpType.logical_shift_left

    with tile.TileContext(nc) as tc:
        with (
            tc.tile_pool(name="big", bufs=1) as big,
            tc.tile_pool(name="psp", bufs=1, space="PSUM") as psp,
            tc.tile_pool(name="wps", bufs=1, space="PSUM") as wps,
            tc.tile_pool(name="outp", bufs=1) as outp,
        ):
            R = big.tile([P, RU * QTOT], fp8)
            S = big.tile([P, UW * QTOT], bf16)
            acc = psp.tile([M, N], f32)

            if n_warm:
                WZ = big.tile([P, WARM_N], bf16)
                wacc = wps.tile([1, WARM_N], f32)
                nc.gpsimd.memset(WZ[:], 0.0)
                for _ in range(n_warm):
                    nc.tensor.matmul(wacc[:, :], WZ[:, 0:1], WZ[:, :],
                                     start=True, stop=True)
            # warm the ACT sigmoid table while the first DMA is in flight
            warm = big.tile([1, 8], bf16)
            nc.gpsimd.memset(warm[:], 0.0)
            nc.scalar.activation(out=warm[:], in_=warm[:], func=sig)

            nch = len(qsizes)
            # Loop A: per chunk, everything that is DMA-gated only — the
            # ones memset (one strided op covers ones_A u64 and ones_B u129,
            # 65 units apart), the input DMA, and the two 4x m-build passes.
            # Emitting the m work here (ahead of all sigmoid consumers) lets
            # DVE burn through it during its early starved phase instead of
            # queuing it between the paced sq/t1 ops.
            qoff = 0
            for c, qs in enumerate(qsizes):
                So = S[:, UW * qoff:UW * (qoff + qs)].rearrange(
                    "p (uu u q) -> p uu u q", uu=2, u=65)
                nc.gpsimd.memset(So[:, :, 64, :], 1.0)
                Sc = S[:, UW * qoff:UW * (qoff + qs)].rearrange(
                    "p (u q) -> p u q", u=UW)
                Rc = R[:, RU * qoff:RU * (qoff + qs)]
                nc.sync.dma_start(Rc, inb[:][:, RU * qoff:RU * (qoff + qs)])
                # mask from x1's fp8 LSBs, via the u16 pair view: two 4x
                # passes write bf16 {0,2} into the halves pixel order that
                # the strided sigmoid also produces.
                pair = Rc[:, 0:B * qs].bitcast(u16).rearrange(
                    "p (u j) -> p u j", u=B)
                mh = Sc[:, 0:B, :].rearrange("p u (t j) -> p u t j", t=2)
                nc.vector.tensor_scalar(
                    mh[:, :, 0, :].bitcast(u16), pair, 1, 14, AND, SHL)
                nc.vector.tensor_scalar(
                    mh[:, :, 1, :].bitcast(u16), pair, 256, 6, AND, SHL)
                qoff += qs
            offs = []
            qoff = 0
            for qs in qsizes:
                offs.append(qoff)
                qoff += qs

            def emit_x1(c):
                # x1 sigmoid + sq1 + t1 for chunk c (tail phase 2)
                qs, qo = qsizes[c], offs[c]
                Sc = S[:, UW * qo:UW * (qo + qs)].rearrange(
                    "p (u q) -> p u q", u=UW)
                Rc = R[:, RU * qo:RU * (qo + qs)]
                m_r = Sc[:, 0:B, :]
                t1_r = Sc[:, B:2 * B, :]
                sq1_r = Sc[:, 65:65 + B, :]
                r1 = Rc[:, 0:B * qs].rearrange(
                    "p (u j t) -> p u t j", u=B, t=2)
                s1 = sq1_r.rearrange("p u (t j) -> p u t j", t=2)
                nc.scalar.activation(out=s1, in_=r1, func=sig)
                kq, kt = sq_pool[c], t1_pool[c]
                if kq:
                    sqa = Sc[:, 65:65 + kq, :]
                    nc.gpsimd.tensor_mul(sqa, sqa, sqa)
                sq1d = Sc[:, 65 + kq:97, :]
                nc.vector.tensor_mul(sq1d, sq1d, sq1d)
                if kt:
                    nc.gpsimd.tensor_mul(t1_r[:, B - kt:B, :],
                                         sq1_r[:, B - kt:B, :],
                                         m_r[:, B - kt:B, :])
                if kt < B:
                    nc.vector.tensor_mul(t1_r[:, 0:B - kt, :],
                                         sq1_r[:, 0:B - kt, :],
                                         m_r[:, 0:B - kt, :])

            def emit_x2_mm(c):
                # x2 sigmoid + sq2 + matmuls for chunk c (tail phase 3)
                qs, qo = qsizes[c], offs[c]
                Sc = S[:, UW * qo:UW * (qo + qs)].rearrange(
                    "p (u q) -> p u q", u=UW)
                Rc = R[:, RU * qo:RU * (qo + qs)]
                r2 = Rc[:, B * qs:2 * B * qs].rearrange(
                    "p (u j t) -> p u t j", u=B, t=2)
                s2 = Sc[:, 97:97 + B, :].rearrange("p u (t j) -> p u t j", t=2)
                nc.scalar.activation(out=s2, in_=r2, func=sig)
                sq2d = Sc[:, 97:129, :]
                nc.vector.tensor_mul(sq2d, sq2d, sq2d)
                for qh in range(qs):
                    q = qo + qh
                    nc.tensor.matmul(
                        acc[:, :], Sc[:, 0:M, qh], Sc[:, M:M + N, qh],
                        start=(q == 0), stop=(q == QTOT - 1))

            for c, qs in enumerate(qsizes):
                qoff = offs[c]
                if c >= split_from:
                    continue
                Sc = S[:, UW * qoff:UW * (qoff + qs)].rearrange(
                    "p (u q) -> p u q", u=UW)
                Rc = R[:, RU * qoff:RU * (qoff + qs)]
                m_r = Sc[:, 0:B, :]
                t1_r = Sc[:, B:2 * B, :]
                sq1_r = Sc[:, 65:65 + B, :]
                sq2_r = Sc[:, 97:97 + B, :]
                kq = sq_pool[c]
                kt = t1_pool[c]
                # fused: x1|x2 sigmoid in ONE ACT op — the input reads fp8
                # pairs strided so the bf16 output lands in the same halves
                # pixel order as the m block (free: ACT charges free-size
                # only, not strides); out spans the adjacent sq1|sq2 blocks.
                rall = Rc[:, 0:RU * qs].rearrange(
                    "p (u j t) -> p u t j", u=RU, t=2)
                sall = Sc[:, 65:129, :].rearrange(
                    "p u (t j) -> p u t j", t=2)
                nc.scalar.activation(out=sall, in_=rall, func=sig)
                if kq:
                    sqa = Sc[:, 65:65 + kq, :]
                    sqb = Sc[:, 129 - kq:129, :]
                    nc.gpsimd.tensor_mul(sqa, sqa, sqa)
                    nc.gpsimd.tensor_mul(sqb, sqb, sqb)
                sqd = Sc[:, 65 + kq:129 - kq, :]
                nc.vector.tensor_mul(sqd, sqd, sqd)
                if kt:
                    nc.gpsimd.tensor_mul(t1_r[:, B - kt:B, :],
                                         sq1_r[:, B - kt:B, :],
                                         m_r[:, B - kt:B, :])
                if kt < B:
                    nc.vector.tensor_mul(t1_r[:, 0:B - kt, :],
                                         sq1_r[:, 0:B - kt, :],
                                         m_r[:, 0:B - kt, :])
                for qh in range(qs):
                    q = qoff + qh
                    nc.tensor.matmul(
                        acc[:, :],
                        Sc[:, 0:M, qh],
                        Sc[:, M:M + N, qh],
                        start=(q == 0),
                        stop=(q == QTOT - 1),
                    )
            # tail: x1 sigmoids (+sq1/t1) for all tail chunks first, then the
            # x2 sigmoids (+sq2/matmuls) — after the final ACT op only one
            # small sq2 remains in the drain.
            for c in range(split_from, nch):
                emit_x1(c)
            for c in range(split_from, nch):
                emit_x2_mm(c)
            res = outp.tile([M, N], f32)
            nc.vector.tensor_copy(res[:], acc[:])
            nc.sync.dma_start(out[:], res[:])
    _batch_matmul_sem_updates(nc)
    _strip_same_engine_sem_waits(nc)
    _strip_out_dma_sync(nc)
    _strip_dead_const_memsets(nc)
    nc.compile()
    return nc


def _strip_same_engine_sem_waits(nc):
    """Tile guards RAW deps between instructions on the SAME engine with that
    engine's completion semaphore, but compute engines execute their stream
    in order (the DVE drains its pipe between ops), so those waits only add
    the producer's pipeline-ack + sem-propagation latency (~95ns per chained
    dep).  Drop any wait on a semaphore that is (a) only ever updated by
    instructions of the waiting instruction's own engine, and (b) already
    guaranteed by program order (wait_value <= updates emitted so far)."""
    blks = nc.m.functions[0].blocks
    owner = {}   # sem id -> set of engines updating it
    for blk in blks:
        for i in blk.instructions:
            si = i.sync_info
            if si is None:
                continue
            for u in si.on_update:
                owner.setdefault(u.id, set()).add(str(i.engine))
    skip = {"InstDMACopy", "InstDMAScatterAddAnt", "InstTriggerDma",
            "InstDmaTransposeAnt"}
    for blk in blks:
        done = {}  # sem id -> updates emitted so far by its (sole) engine
        for i in blk.instructions:
            si = i.sync_info
            if si is None:
                continue
            eng = str(i.engine)
            if type(i).__name__ not in skip:
                new_waits = []
                for w in si.on_wait:
                    own = owner.get(w.id)
                    if (own == {eng} and w.wait_value is not None
                            and w.wait_value <= done.get(w.id, 0)):
                        continue
                    new_waits.append(w)
                if len(new_waits) != len(si.on_wait):
                    si.on_wait = new_waits
                    i.sync_info = si
            for u in si.on_update:
                if owner.get(u.id) == {eng} and u.update_value is not None:
                    done[u.id] = done.get(u.id, 0) + u.update_value


def _strip_dead_const_memsets(nc):
    blk = nc.m.functions[0].blocks[0]
    names = set()
    for b in nc.m.functions[0].blocks:
        for inst in b.instructions:
            for ap in list(inst.ins):
                s = str(ap)
                for n in ("const-float32-0.0", "const-float32-1.0",
                          "const-bfloat16-1.0", "const-uint8-127"):
                    if n in s:
                        names.add(n)
    keep = []
    for inst in blk.instructions:
        if (type(inst).__name__ == "InstMemset" and inst.sync_info is None
                and list(inst.outs)):
            s = str(inst.outs[0])
            if "const-" in s and not any(n in s for n in names):
                continue
        keep.append(inst)
    blk.instructions = keep
    _strip_entry_barrier(nc)


def _strip_entry_barrier(nc):
    blk = nc.m.functions[0].blocks[0]
    barrier_ids = set()
    for inst in blk.instructions:
        if type(inst).__name__ in ("InstDrain", "InstEventSemaphore"):
            si = inst.sync_info
            if si:
                barrier_ids |= {w.id for w in si.on_wait}
                barrier_ids |= {u.id for u in si.on_update}
    keep = []
    for inst in blk.instructions:
        if type(inst).__name__ in ("InstDrain", "InstEventSemaphore"):
            si = inst.sync_info
            ids = set()
            if si:
                ids |= {w.id for w in si.on_wait}
                ids |= {u.id for u in si.on_update}
            if ids and ids <= barrier_ids:
                continue
            if si is None and type(inst).__name__ == "InstDrain":
                continue
        keep.append(inst)
    blk.instructions = keep


def _strip_out_dma_sync(nc):
    blks = nc.m.functions[0].blocks
    last_dma = None
    for blk in blks:
        for i in blk.instructions:
            if type(i).__name__ == "InstDMACopy":
                last_dma = i
    si = last_dma.sync_info
    if si is None or len(si.on_update) != 1:
        return
    upd = si.on_update[0]
    sem_id, val = upd.id, upd.update_value
    total = 0
    for blk in blks:
        for i in blk.instructions:
            s2 = i.sync_info
            if s2 is None:
                continue
            for u in s2.on_update:
                if u.id == sem_id:
                    total += u.update_value
    for blk in blks:
        for i in blk.instructions:
            s2 = i.sync_info
            if s2 is None or i is last_dma:
                continue
            changed = False
            for w in s2.on_wait:
                if w.id == sem_id and w.wait_value == total:
                    w.wait_value = total - val
                    changed = True
            if changed:
                i.sync_info = s2


def _batch_matmul_sem_updates(nc):
    for blk in nc.m.functions[0].blocks:
        mms = [i for i in blk.instructions if type(i).__name__ == "InstMatmult"]
        if not mms:
            continue
        total = 0
        sem_id = None
        for i in mms:
            si = i.sync_info
            if si is None:
                continue
            for u in si.on_update:
                assert u.update_mode == "sem-inc"
                sem_id = u.id
                total += u.update_value
        kept = 0
        for i in mms[:-1]:
            si = i.sync_info
            if si is None:
                continue
            if len(si.on_wait) == 0 and len(si.on_update) == 1:
                i.sync_info = None
            else:
                kept += sum(u.update_value for u in si.on_update
                            if u.id == sem_id)
        kept += 1
        for blk2 in nc.m.functions[0].blocks:
            for i in blk2.instructions:
                si = i.sync_info
                if si is None:
                    continue
                changed = False
                for w in si.on_wait:
                    if w.id == sem_id and w.wait_value == total:
                        w.wait_value = kept
                        changed = True
                if changed:
                    i.sync_info = si


def _get_nc():
    if "nc" not in _CACHE:
        _CACHE["nc"] = _build()
    return _CACHE["nc"]


_F8LUT = None


def _f8_lut():
    global _F8LUT
    if _F8LUT is None:
        import ml_dtypes
        _F8LUT = np.arange(256, dtype=np.uint8).view(
            ml_dtypes.float8_e4m3fn).astype(np.float32)
    return _F8LUT


def _to_fp8_mask_lsb(x, mask_bits):
    """fp8-e4m3 bytes for x with the LSB forced to mask_bits, choosing the
    closer of the two LSB-matching neighbours."""
    import ml_dtypes
    lut = _f8_lut()
    a = x.astype(ml_dtypes.float8_e4m3fn).view(np.uint8)
    c0 = (a & np.uint8(0xFE)) | mask_bits
    # alternate candidate two code-points away (same LSB), toward x
    v0 = lut[c0]
    mag_up = (np.abs(x) > np.abs(v0))
    step = np.where(mag_up, 2, -2).astype(np.int16)
    c1 = (c0.astype(np.int16) + step)
    # keep sign bit intact; clamp to valid magnitude range
    bad = (c1 & 0x7F) < 0
    c1 = np.where((c1 ^ c0.astype(np.int16)) & 0x80, c0.astype(np.int16), c1)
    c1 = np.where(bad, c0.astype(np.int16), c1).astype(np.uint8)
    v1 = lut[c1]
    pick1 = np.abs(v1 - x) < np.abs(v0 - x)
    return np.where(pick1, c1, c0).astype(np.uint8)


def _stage(u1, u2, qsizes=None):
    """Two [B, HWTOT] u8 (fp8-byte) arrays -> per-core [P, 2B*QTOT] chunk-
    major: per chunk the x1 block then the x2 block."""
    qsizes = QSIZES if qsizes is None else qsizes
    v1 = u1.reshape(B, NCORES, P, QTOT)
    v2 = u2.reshape(B, NCORES, P, QTOT)
    parts = []
    qoff = 0
    for qs in qsizes:
        k1 = v1[..., qoff:qoff + qs].transpose(1, 2, 0, 3)  # [NC, P, B, qs]
        k2 = v2[..., qoff:qoff + qs].transpose(1, 2, 0, 3)
        parts.append(k1.reshape(NCORES, P, B * qs))
        parts.append(k2.reshape(NCORES, P, B * qs))
        qoff += qs
    outv = np.ascontiguousarray(np.concatenate(parts, axis=2))
    return [outv[k] for k in range(NCORES)]


LAST_RESULT = None


def kernel(input1, input2, mask):
    import ml_dtypes
    from concourse.bass_utils import run_bass_kernel_spmd

    global LAST_RESULT
    x1 = np.asarray(input1, dtype=np.float32).reshape(B, HWTOT)
    x2 = np.asarray(input2, dtype=np.float32).reshape(B, HWTOT)
    mk = (np.asarray(mask, dtype=np.float32).reshape(B, HWTOT) != 0)

    u1 = _to_fp8_mask_lsb(x1, mk.astype(np.uint8))
    u2 = x2.astype(ml_dtypes.float8_e4m3fn).view(np.uint8)

    sb = _stage(u1, u2)
    in_maps = [{"inb": sb[k].view(ml_dtypes.float8_e4m3fn)}
               for k in range(NCORES)]
    nc = _get_nc()
    trace = bool(int(os.environ.get("BASSKERNEL_TRACE", "0")))
    try:
        res = run_bass_kernel_spmd(
            nc, in_maps, core_ids=list(range(NCORES)), trace=trace,
        )
    except ModuleNotFoundError:
        res = run_bass_kernel_spmd(
            nc, in_maps, core_ids=list(range(NCORES)), trace=False,
        )
    LAST_RESULT = res

    Ms = np.zeros((M, N), dtype=np.float64)
    for r in res.results:
        Ms += np.asarray(r["out"], dtype=np.float64)

    # stationary rows: m(0..31) | t1(32..63) | ones_A(64) | sq1(65..96)
    # moving cols:     sq2(0..31) | ones_B(32)
    d2 = 0.5 * np.diag(Ms[0:B, 0:B])
    pn = 0.5 * np.diag(Ms[B:2 * B, 0:B])
    d1 = 0.5 * Ms[B:2 * B, B]
    s2v = Ms[64, 0:B]
    G = Ms[65:65 + B, 0:B]
    s1v = Ms[65:65 + B, B]

    sim_pos = np.sqrt(pn) / (np.sqrt(d1) * np.sqrt(d2))          # [B]
    sim = np.sqrt(G) / (np.sqrt(s1v)[:, None] * np.sqrt(s2v)[None, :])
    sim_neg = sim.sum(axis=1) - np.diag(sim)                      # [B]
    ratio = sim_pos[None, :] / (sim_pos[None, :] + sim_neg[:, None])
    loss = -np.log(ratio)
    return np.array(loss.mean(), dtype=np.float32)
